# revision 8
# baseline (speedup 1.0000x reference)
"""nn_BeamSearch Trainium2 kernel.

Architecture (per spec sharding_hint): the 32k-vocab output projection — the
memory-bound bulk of the model (131 MB of out_W traffic per decode step) — is
tensor-parallel sharded over the vocab dim across 8 NeuronCores and computed by
a Bass kernel (fp32 PE matmuls, weights streamed HBM->SBUF, K-chunk PSUM
accumulation matching XLA's dot lowering bit-for-bit). The tiny recurrent
state math (2-layer GRU cells, dot attention, log_softmax/top_k selection and
the beam probability chain) runs between projection launches via small jitted
jax ops on the same NeuronCores, preserving the reference's exact arithmetic
(same ACT tables / accumulation orders), which matters because the beam
probability products underflow into fp32 subnormals by step ~9 and the final
top_k decisions hinge on single-quantum comparisons.

The Bass NEFF is compiled once and launched once per decode step (11 total);
the sharded out_W stays device-resident across launches.
"""

import numpy as np

V = 32000
D = 512
L = 2
WIDTH = 4
MAXLEN = 12
NCORES = 8
VS = V // NCORES          # 4000 vocab columns per core
NK = 9                    # 8 K-chunks of 128 (d=1024) + 1 bias chunk
NB = 8                    # 500-wide N tiles per core (one PSUM bank each)
NW = 500


def _build_projection_kernel():
    """Bass kernel: logits[4, VS] = y[4, 1024] @ Wshard[1024, VS] + b_shard.

    Inputs per core:
      yT [NK, 128, 4]  — y transposed into K-chunks (stationary operand);
                          chunk 8 is the bias trick: row0 = ones.
      Wp [NK, 128, VS] — weight shard in K-chunks; chunk 8 row0 = b shard.
    Output per core:
      logits [4, VS]
    """
    import concourse.bacc as bacc
    import concourse.mybir as mybir
    from concourse.tile import TileContext

    f32 = mybir.dt.float32
    nc = bacc.Bacc("TRN2", target_bir_lowering=False, debug=False)
    yT = nc.dram_tensor("yT", [NK, 128, WIDTH], f32, kind="ExternalInput")
    Wp = nc.dram_tensor("Wp", [NK, 128, VS], f32, kind="ExternalInput")
    out = nc.dram_tensor("logits", [WIDTH, VS], f32, kind="ExternalOutput")

    with TileContext(nc) as tc:
        with (
            tc.tile_pool(name="ypool", bufs=1) as ypool,
            tc.tile_pool(name="wspool", bufs=8) as wspool,
            tc.tile_pool(name="wpool", bufs=2) as wpool,
            tc.tile_pool(name="opool", bufs=1) as opool,
            tc.tile_pool(name="psum", bufs=1, space="PSUM") as ppool,
        ):
            # All matmul operand tiles are produced by DVE copies: the fused
            # LDW+MM instruction only supports a single sync wait, and raw
            # HWDGE DMAs fan out to multiple queue semaphores. Staging the
            # DMA through a DVE copy funnels every dependency into one
            # program-ordered DVE tick.
            ys = ypool.tile([128, NK, WIDTH], f32, tag="ys")
            nc.gpsimd.dma_start(ys[:, :, :], yT.ap().rearrange("k p f -> p k f"))
            yt = ypool.tile([128, NK, WIDTH], f32, tag="yt")
            nc.vector.tensor_copy(yt[:, :, :], ys[:, :, :])
            psum = ppool.tile([WIDTH, NB, 512], f32)
            for kc in range(NK):
                ws = wspool.tile([128, VS], f32, tag="ws")
                nc.gpsimd.dma_start(ws[:, :], Wp[kc, :, :])
                wt = wpool.tile([128, VS], f32, tag="w")
                nc.vector.tensor_copy(wt[:, :], ws[:, :])
                for nb in range(NB):
                    nc.tensor.matmul(
                        psum[:, nb, 0:NW],
                        yt[:, kc, :],
                        wt[:, nb * NW:(nb + 1) * NW],
                        start=(kc == 0),
                        stop=(kc == NK - 1),
                    )
            lt = opool.tile([WIDTH, VS], f32)
            nc.scalar.copy(lt[:, :].rearrange("p (b w) -> p b w", b=NB),
                           psum[:, :, 0:NW])
            nc.gpsimd.dma_start(out[:, :], lt[:, :])
    nc.compile()
    return nc


class _ProjRunner:
    """Compile-once, run-many SPMD executor for the projection NEFF.

    Adapted from concourse.bass2jax.run_bass_via_pjrt, with the jitted body
    hoisted so repeated launches reuse the compiled executable and the
    device-resident sharded weights.
    """

    def __init__(self, W_all):
        import jax
        from jax.sharding import Mesh, PartitionSpec, NamedSharding
        from jax.experimental.shard_map import shard_map
        from concourse.bass2jax import (
            install_neuronx_cc_hook, _bass_exec_p, partition_id_tensor,
        )
        import concourse.mybir as mybir

        install_neuronx_cc_hook()
        nc = _build_projection_kernel()
        self._jax = jax

        partition_name = (nc.partition_id_tensor.name
                          if nc.partition_id_tensor else None)
        in_names, out_names, out_avals = [], [], []
        for alloc in nc.m.functions[0].allocations:
            if not isinstance(alloc, mybir.MemoryLocationSet):
                continue
            name = alloc.memorylocations[0].name
            if alloc.kind == "ExternalInput":
                if name != partition_name:
                    in_names.append(name)
            elif alloc.kind == "ExternalOutput":
                out_names.append(name)
                out_avals.append(jax.core.ShapedArray(
                    tuple(alloc.tensor_shape), mybir.dt.np(alloc.dtype)))
        assert in_names == ["yT", "Wp"] and out_names == ["logits"], (in_names, out_names)
        n_params, n_outs = len(in_names), len(out_avals)
        all_names = in_names + out_names
        if partition_name is not None:
            all_names = all_names + [partition_name]

        def _body(*args):
            operands = list(args)
            if partition_name is not None:
                operands.append(partition_id_tensor())
            outs = _bass_exec_p.bind(
                *operands,
                out_avals=tuple(out_avals),
                in_names=tuple(all_names),
                out_names=tuple(out_names),
                lowering_input_output_aliases=(),
                sim_require_finite=True,
                sim_require_nnan=True,
                nc=nc,
            )
            return tuple(outs)

        devices = jax.devices()[:NCORES]
        assert len(devices) == NCORES, f"need {NCORES} cores, got {len(jax.devices())}"
        mesh = Mesh(np.asarray(devices), ("core",))
        self._exec = jax.jit(
            shard_map(
                _body, mesh=mesh,
                in_specs=(PartitionSpec("core"),) * (n_params + n_outs),
                out_specs=(PartitionSpec("core"),) * n_outs,
                check_rep=False,
            ),
            donate_argnums=(n_params,),
            keep_unused=True,
        )
        # W device-resident, sharded along concat axis 0 (NK rows per core).
        self._Wg = jax.device_put(
            W_all, NamedSharding(mesh, PartitionSpec("core")))

    def __call__(self, y):
        """y: [4, 1024] fp32 -> logits [4, 32000] fp32 (host numpy)."""
        yT = np.ascontiguousarray(
            y.T.reshape(8, 128, WIDTH).astype(np.float32))      # [8,128,4]
        bias_chunk = np.zeros((1, 128, WIDTH), np.float32)
        bias_chunk[0, 0, :] = 1.0
        yT = np.concatenate([yT, bias_chunk], axis=0)           # [NK,128,4]
        yT_g = np.concatenate([yT] * NCORES, axis=0)            # same y on all cores
        zeros = np.zeros((NCORES * WIDTH, VS), np.float32)
        (lg,) = self._exec(yT_g, self._Wg, zeros)
        lg = np.asarray(lg).reshape(NCORES, WIDTH, VS)
        return np.concatenate([lg[c] for c in range(NCORES)], axis=1)  # [4, V]


def kernel(src, src_len, width, max_len,
           enc_emb, enc_Wih, enc_Whh, enc_bih, enc_bhh,
           dec_emb, dec_Wih, dec_Whh, dec_bih, dec_bhh,
           out_W, out_b):
    import jax
    import jax.numpy as jnp

    src = np.asarray(src)
    out_W = np.asarray(out_W, np.float32)
    out_b = np.asarray(out_b, np.float32)

    # ---- shard out_W over vocab across 8 cores, K-chunked, bias folded ----
    W_shards = []
    for c in range(NCORES):
        ws = out_W[:, c * VS:(c + 1) * VS]                       # [1024, VS]
        wk = ws.reshape(8, 128, VS)                              # K-chunks
        bc = np.zeros((1, 128, VS), np.float32)
        bc[0, 0, :] = out_b[c * VS:(c + 1) * VS]
        W_shards.append(np.concatenate([wk, bc], axis=0))        # [NK,128,VS]
    W_all = np.ascontiguousarray(np.concatenate(W_shards, axis=0))  # [NK*8,128,VS]
    proj = _ProjRunner(W_all)

    # ---- small recurrent math on-device via jax (reference-exact ops) ----
    enc_emb = jnp.asarray(enc_emb, jnp.float32)
    dec_emb = jnp.asarray(dec_emb, jnp.float32)
    enc_Wih, enc_Whh = jnp.asarray(enc_Wih), jnp.asarray(enc_Whh)
    enc_bih, enc_bhh = jnp.asarray(enc_bih), jnp.asarray(enc_bhh)
    dec_Wih, dec_Whh = jnp.asarray(dec_Wih), jnp.asarray(dec_Whh)
    dec_bih, dec_bhh = jnp.asarray(dec_bih), jnp.asarray(dec_bhh)

    def _gru(x, h, Wih, Whh, bih, bhh):
        gi = x @ Wih + bih
        gh = h @ Whh + bhh
        ir, iz, inew = jnp.split(gi, 3, axis=-1)
        hr, hz, hn = jnp.split(gh, 3, axis=-1)
        r = jax.nn.sigmoid(ir + hr)
        z = jax.nn.sigmoid(iz + hz)
        n = jnp.tanh(inew + r * hn)
        return (1.0 - z) * n + z * h

    @jax.jit
    def run_encoder(src_):
        x = enc_emb[src_[:, 0]]
        hs = []
        for l in range(L):
            def step(h, xt, l=l):
                h2 = _gru(xt, h, enc_Wih[l], enc_Whh[l], enc_bih[l], enc_bhh[l])
                return h2, h2
            hT, x = jax.lax.scan(step, jnp.zeros((D,), x.dtype), x)
            hs.append(hT)
        return x, jnp.stack(hs)          # enc_out [S,D], enc_h [L,D]

    def _dec_small(tok, h, enc_out):
        xt = dec_emb[tok]
        new_h = []
        for l in range(L):
            hl = _gru(xt, h[l], dec_Wih[l], dec_Whh[l], dec_bih[l], dec_bhh[l])
            new_h.append(hl)
            xt = hl
        top = xt
        attn = jax.nn.softmax(enc_out @ top)
        ctx = attn @ enc_out
        return jnp.concatenate([top, ctx]), jnp.stack(new_h)

    dec_small = jax.jit(jax.vmap(_dec_small, in_axes=(0, 0, None)))

    @jax.jit
    def sel(logits):                      # per-beam log_softmax + top-k
        lp = jax.nn.log_softmax(logits)
        return jax.lax.top_k(lp, WIDTH)

    @jax.jit
    def score_step(val1, prob_best):      # reference's subnormal prob chain
        prob_options = (jnp.exp(val1) * prob_best[:, None]).reshape(-1)
        pb, top_idx = jax.lax.top_k(prob_options, WIDTH)
        return pb, top_idx

    @jax.jit
    def init_probs(val):
        return jnp.exp(val)

    # ---- beam search drive loop ----
    enc_out, enc_h = run_encoder(jnp.asarray(src))

    s00 = jnp.asarray(src)[0, 0].astype(jnp.int32)
    toks0 = jnp.full((WIDTH,), s00, jnp.int32)
    h0 = jnp.broadcast_to(enc_h[None], (WIDTH, L, D))
    y, _ = dec_small(toks0, h0, enc_out)
    logits0 = proj(np.asarray(y))                       # bass launch 1
    val, idx = sel(jnp.asarray(logits0))
    val, idx = val[0], idx[0]                           # init used beam 0 only
    prob_best = init_probs(val)

    beam_best = np.zeros((WIDTH, MAXLEN), np.float32)
    beam_best[:, 1] = np.asarray(idx, np.float32)

    # state after consuming token 0 (shared by all beams), then beam tokens
    toksz = jnp.zeros((WIDTH,), jnp.int32)
    _, h_common = dec_small(toksz, h0, enc_out)
    h_common = h_common[0]
    y, h_beam = dec_small(idx.astype(jnp.int32),
                          jnp.broadcast_to(h_common[None], (WIDTH, L, D)), enc_out)
    logits = proj(np.asarray(y))                        # bass launch 2

    beam_options = np.zeros((WIDTH * WIDTH, MAXLEN), np.float32)
    for t in range(2, MAXLEN):
        val1, idx1 = sel(jnp.asarray(logits))           # [4,4] each
        beam_options = np.repeat(beam_best, WIDTH, axis=0)
        beam_options[:, t] = np.asarray(idx1).reshape(-1).astype(np.float32)
        prob_best, top_idx = score_step(val1, prob_best)
        top_idx_np = np.asarray(top_idx)
        beam_best = beam_options[top_idx_np]
        if t < MAXLEN - 1:
            parents = top_idx_np // WIDTH
            toks = jnp.asarray(beam_best[:, t].astype(np.int32))
            h_parent = jnp.asarray(np.asarray(h_beam)[parents])
            y, h_beam = dec_small(toks, h_parent, enc_out)
            logits = proj(np.asarray(y))                # bass launches 3..11

    best = int(np.argmax(np.asarray(prob_best)))
    return beam_options[best].astype(np.float32)


if __name__ == "__main__":
    _c = np.load("/root/problem/cache_ref.npz")
    ins = {k: _c[k] for k in _c.files if k != "expected"}
    out = kernel(**ins)
    print("kernel:", out)
    print("expect:", _c["expected"])
    print("MATCH:", np.array_equal(out, _c["expected"]))
